# revision 8
# baseline (speedup 1.0000x reference)
"""Trainium2 Bass kernel for nn_CrossAttention_50070728737309.

Reference computation (see problem spec):
  q  = conv1x1(rms_norm(x, q_norm_w),  q_w,  q_b)    x: [4, 256, 48, 48] -> q: [4, 128, 48, 48]
  kv = conv1x1(rms_norm(y, kv_norm_w), kv_w, kv_b)   y: [4, 512, 48, 48] -> kv: [4, 256, 48, 48]
  4-head attention over N = 48*48 = 2304 positions, head_dim 32
  out = conv1x1(attn_out, proj_w, proj_b)            [4, 256, 48, 48]

Sharding: 8 cores = (batch b in 0..3) x (query-half in 0..1). Each core gets
its batch's full y (keys/values attend over all 2304 positions) and a
1152-column slice of x (query positions). Outputs concatenate on the host --
no cross-core reduction needed.

Device-side algorithm (per core), v2 -- no GPSIMD instructions at all:
  - norm weights pre-multiplied into q_w/kv_w on host. Per-position
    inv_rms computed via ones-matmul column sums.
  - k bias dropped entirely: softmax is shift-invariant along the key
    axis, so the (k_b . q)[n] term cancels exactly.
  - v bias folded into proj bias on host (pb_eff).
  - y-side inv_rms kept in COLUMN layout [128, MT] (one transpose-style
    K=1 matmul per m-tile) and folded into the softmax exp as a
    per-partition ACT scale -> k needs NO normalization multiply, and
    v's normalization rides the transpose-evacuation tensor_scalar_mul.
  - x-side inv_rms stays a row [1, NH]; broadcast via K=1 ones-matmul on
    the PE (not gpsimd), multiplied into q on evacuation.
  - S^T layout attention: S^T[m, n] = k_h^T q_h via 4-way row-tiled (K=32)
    matmuls, all 4 heads concurrent in the PE array, into one bank-strided
    PSUM quad tile. exp on ACT with scale = SCALE * inv_rms_y[m].
  - PV: lhsT = [v_h^T | ones] (33 cols) so each matmul also accumulates the
    softmax denominator as row 32; 2 heads packed per PSUM bank via 64-column
    PE tiling. Reciprocal of the denominator broadcast across 32 partitions
    via K=1 ones-matmul into a spare PSUM quad; division on DVE.
  - proj: K=128 matmul over the stacked head outputs; proj_b added on evac.

Matmul dtype float32r (~1e-3 rel err, 4x faster than fp32 on the PE).
"""

import os
import sys

import numpy as np

# The harness may run this from a bare directory; concourse comes from the
# container's PYTHONPATH. Add the known locations as a fallback.
for _p in ("/root/.axon_site", "/root/.axon_site/_ro/trn_rl_repo",
           "/root/.axon_site/_ro/pypackages", "/opt/trn_rl_repo"):
    if _p not in sys.path and os.path.isdir(_p):
        sys.path.append(_p)

B = 4
CQ = 256
CKV = 512
N = 2304          # 48*48 positions
NH = N // 2       # query positions per core
DIM = 128
HEADS = 4
HD = 32
EPS = 1.5e-5
SCALE = HD ** -0.5
NW = 384          # attention n-window (3 windows of 384 = 1152)
NWIN = NH // NW
MT = N // 128     # 18 m-tiles
N_CORES = 8

_EXEC = None  # cached compiled executor


def _build_module(reps=1):
    from contextlib import ExitStack

    import concourse.tile as tile
    from concourse import bacc, mybir
    from concourse.masks import make_identity

    F32 = mybir.dt.float32
    F32R = mybir.dt.float32r
    BF16 = mybir.dt.bfloat16
    AF = mybir.ActivationFunctionType

    nc = bacc.Bacc("TRN2", target_bir_lowering=False, debug=False,
                   num_devices=N_CORES)

    xb = nc.dram_tensor("xb", [2, 128, NH], F32, kind="ExternalInput").ap()
    yb = nc.dram_tensor("yb", [4, 128, N], F32, kind="ExternalInput").ap()
    qwT = nc.dram_tensor("qwT", [2, 128, 128], F32, kind="ExternalInput").ap()
    kvwT = nc.dram_tensor("kvwT", [4, 128, 256], F32, kind="ExternalInput").ap()
    pwT = nc.dram_tensor("pwT", [128, 256], F32, kind="ExternalInput").ap()
    qb_d = nc.dram_tensor("qb", [128, 1], F32, kind="ExternalInput").ap()
    pb_d = nc.dram_tensor("pb", [2, 128, 1], F32, kind="ExternalInput").ap()
    ones_d = nc.dram_tensor("ones", [128, 1], F32, kind="ExternalInput").ap()
    o_d = nc.dram_tensor("o", [2, 128, NH], F32, kind="ExternalOutput").ap()

    with tile.TileContext(nc) as tc, ExitStack() as ctx, \
            nc.allow_low_precision(reason="float32r rounding is intentional"):
        consts = ctx.enter_context(tc.tile_pool(name="consts", bufs=1))

        ident = consts.tile([128, 128], F32)
        make_identity(nc, ident)
        qw_sb = consts.tile([128, 2, 128], F32R)
        nc.sync.dma_start(qw_sb[:], qwT.bitcast(F32R).rearrange("t p n -> p t n"))
        kvw_sb = consts.tile([128, 4, 256], F32R)
        nc.sync.dma_start(kvw_sb[:], kvwT.bitcast(F32R).rearrange("t p n -> p t n"))
        pw_sb = consts.tile([128, 256], F32R)
        nc.sync.dma_start(pw_sb[:], pwT.bitcast(F32R))
        qb_sb = consts.tile([128, 1], F32)
        nc.sync.dma_start(qb_sb[:], qb_d[:])
        pb_sb = consts.tile([128, 2], F32)
        nc.sync.dma_start(pb_sb[:], pb_d.rearrange("t p n -> p (t n)"))
        ones_sb = consts.tile([128, 1], F32R)
        nc.sync.dma_start(ones_sb[:], ones_d.bitcast(F32R))
        ones_row = consts.tile([1, 128], F32R)
        nc.sync.dma_start(ones_row[:], ones_d.bitcast(F32R).rearrange("p n -> n p"))
        eps_sb = consts.tile([1, 1], F32)
        nc.vector.memset(eps_sb[:], EPS)
        epsy_sb = consts.tile([128, 1], F32)
        nc.vector.memset(epsy_sb[:], EPS / (SCALE * SCALE))

        for _rep in range(reps):
          with tc.tile_pool(name=f"persist{_rep}", bufs=1) as persist:
              # persistent across stages
              q_sb = persist.tile([128, NH], F32R)
              k_sb = persist.tile([128, N], F32R)
              vT_aug = persist.tile([128, MT, HEADS, 34], BF16)
              attn_sb = persist.tile([128, NH], F32R)
              o_sb = persist.tile([128, 2, NH], F32)
              # column-pair layout [128, 2*MT]; even columns are used
              invy_s = persist.tile([128, 2 * MT], F32)   # SCALE * inv_rms_y
              invy_p = persist.tile([128, 2 * MT], F32)   # inv_rms_y

              # ---------------- stage 1: norms + projections + v^T ----------------
              with ExitStack() as s1:
                  big = s1.enter_context(tc.tile_pool(name=f"big{_rep}", bufs=1))
                  sq = s1.enter_context(tc.tile_pool(name=f"sq{_rep}", bufs=2))
                  inv = s1.enter_context(tc.tile_pool(name=f"inv{_rep}", bufs=1))
                  ps_ss = s1.enter_context(tc.tile_pool(name=f"ps_ss{_rep}", bufs=1, space="PSUM"))
                  ps_mm = s1.enter_context(tc.tile_pool(name=f"ps_mm{_rep}", bufs=2, space="PSUM"))
                  ps_kv = s1.enter_context(tc.tile_pool(name=f"ps_kv{_rep}", bufs=2, space="PSUM"))
                  ps_t = s1.enter_context(tc.tile_pool(name=f"ps_t{_rep}", bufs=2, space="PSUM"))
                  ps_bc = s1.enter_context(tc.tile_pool(name=f"ps_bc{_rep}", bufs=1, space="PSUM"))

                  x_t = [big.tile([128, NH], F32R, name=f"x{t}", bufs=1) for t in range(2)]
                  y_t = [big.tile([128, N], F32R, name=f"y{t}", bufs=1) for t in range(4)]
                  for t in range(4):
                      for j in range(4):
                          j0, j1 = j * 576, (j + 1) * 576
                          eng = nc.sync if (t + j) % 2 == 0 else nc.scalar
                          eng.dma_start(y_t[t][:, j0:j1], yb.bitcast(F32R)[t][:, j0:j1])
                  for t in range(2):
                      for j in range(2):
                          j0, j1 = j * 576, (j + 1) * 576
                          eng = nc.sync if (t + j) % 2 == 0 else nc.scalar
                          eng.dma_start(x_t[t][:, j0:j1], xb.bitcast(F32R)[t][:, j0:j1])

                  # --- x-side inv_rms row: 1/sqrt(mean(x^2)+eps) in [1, NH] ---
                  ssx = inv.tile([1, NH], F32R, name="ssx", bufs=1)
                  for j in range(3):
                      j0, j1 = j * 512, min((j + 1) * 512, NH)
                      ps = ps_ss.tile([1, 512], F32, name="ss_ps", tag="ss_ps")
                      for t, xt in enumerate(x_t):
                          x2 = sq.tile([128, 512], F32R, name="x2", tag="x2")
                          if (t + j) % 2 == 0:
                              nc.scalar.activation(out=x2[:, 0:j1 - j0],
                                                   in_=xt[:, j0:j1], func=AF.Square)
                          else:
                              nc.vector.tensor_mul(x2[:, 0:j1 - j0], xt[:, j0:j1],
                                                   xt[:, j0:j1])
                          nc.tensor.matmul(
                              out=ps[0:1, 0:j1 - j0], lhsT=ones_sb[:],
                              rhs=x2[:, 0:j1 - j0],
                              start=(t == 0), stop=(t == 1))
                      nc.scalar.activation(out=ssx[0:1, j0:j1], in_=ps[0:1, 0:j1 - j0],
                                           func=AF.Sqrt, scale=1.0 / CQ,
                                           bias=eps_sb[:])
                      nc.vector.reciprocal(ssx[0:1, j0:j1], ssx[0:1, j0:j1])

                  # --- y-side sum of squares row [1, N], then to columns ---
                  ssq = inv.tile([1, N], F32R, name="ssq", bufs=1)
                  for j in range(5):
                      j0, j1 = j * 512, min((j + 1) * 512, N)
                      ps = ps_ss.tile([1, 512], F32, name="ss_ps", tag="ss_ps")
                      for t, yt in enumerate(y_t):
                          y2 = sq.tile([128, 512], F32R, name="x2", tag="x2")
                          if (t + j) % 2 == 0:
                              nc.scalar.activation(out=y2[:, 0:j1 - j0],
                                                   in_=yt[:, j0:j1], func=AF.Square)
                          else:
                              nc.vector.tensor_mul(y2[:, 0:j1 - j0], yt[:, j0:j1],
                                                   yt[:, j0:j1])
                          nc.tensor.matmul(
                              out=ps[0:1, 0:j1 - j0], lhsT=ones_sb[:],
                              rhs=y2[:, 0:j1 - j0],
                              start=(t == 0), stop=(t == 3))
                      nc.vector.tensor_copy(ssq[0:1, j0:j1], ps[0:1, 0:j1 - j0])

                  # transpose ssq row -> [128, MT] columns via K=1 matmuls
                  ssqT = ps_bc.tile([128, 512], F32, name="bc", tag="bc")
                  for mt in range(MT):
                      nc.tensor.matmul(
                          out=ssqT[:, 2 * mt:2 * mt + 2],
                          lhsT=ssq[0:1, mt * 128:(mt + 1) * 128],
                          rhs=ones_row[0:1, 0:2], start=True, stop=True)
                  # sqrt((ssq/C + eps)/SCALE^2) so reciprocal lands at SCALE*inv
                  sy = inv.tile([128, 2 * MT], F32, name="sy", bufs=1)
                  nc.scalar.activation(out=sy[:], in_=ssqT[:, 0:2 * MT], func=AF.Sqrt,
                                       scale=1.0 / (CKV * SCALE * SCALE),
                                       bias=epsy_sb[:])
                  nc.vector.reciprocal(invy_s[:], sy[:])
                  nc.vector.tensor_scalar_mul(invy_p[:], invy_s[:], 1.0 / SCALE)

                  # --- broadcast ssx across partitions: PE K=1 matmul + ACT evac ---
                  bcx = inv.tile([128, NH], F32, name="bcx", bufs=1)
                  for j in range(NWIN):
                      sl = slice(j * NW, (j + 1) * NW)
                      bc = ps_bc.tile([128, 512], F32, name="bc", tag="bc")
                      nc.tensor.matmul(out=bc[:, 0:NW], lhsT=ones_row[:],
                                       rhs=ssx[0:1, sl],
                                       start=True, stop=True)
                      nc.scalar.activation(out=bcx[:, sl], in_=bc[:, 0:NW],
                                           func=AF.Copy)

                  # --- q = (q_w_eff @ x) * bcx + q_b ---
                  for j in range(NWIN):
                      ps = ps_mm.tile([128, NW], F32, name="qpre", tag="qpre")
                      sl = slice(j * NW, (j + 1) * NW)
                      for t in range(2):
                          nc.tensor.matmul(out=ps[:], lhsT=qw_sb[:, t, :],
                                           rhs=x_t[t][:, sl],
                                           start=(t == 0), stop=(t == 1))
                      nc.vector.tensor_mul(q_sb[:, sl], ps[:], bcx[:, sl])
                      nc.vector.tensor_scalar_add(q_sb[:, sl], q_sb[:, sl], qb_sb[:])

                  # --- k, v = kv_w_eff @ y (normalization deferred) ---
                  nc.vector.memset(vT_aug[:], 1.0)
                  v_sb = big.tile([128, N], F32)
                  for m, dst in enumerate([k_sb, v_sb]):
                      for j in range(6):
                          sl = slice(j * 384, (j + 1) * 384)
                          ps = ps_kv.tile([128, 384], F32, name="kvpre", tag="kvpre")
                          for t in range(4):
                              nc.tensor.matmul(
                                  out=ps[:], lhsT=kvw_sb[:, t, m * 128:(m + 1) * 128],
                                  rhs=y_t[t][:, sl], start=(t == 0), stop=(t == 3))
                          nc.vector.tensor_copy(dst[:, sl], ps[:])
                          if m == 1:
                              # v chunk ready: transpose its 3 m-tiles, evac with
                              # the inv_rms_y scale fused (per-partition scalar)
                              for mt in range(3 * j, 3 * j + 3):
                                  ps2 = ps_t.tile([128, 128], F32, name="tps", tag="tps")
                                  nc.tensor.transpose(
                                      ps2[:], v_sb[:, mt * 128:(mt + 1) * 128], ident[:])
                                  nc.vector.tensor_scalar_mul(
                                      vT_aug[:, mt, :, 0:32],
                                      ps2[:, :].rearrange("p (h d) -> p h d", h=HEADS),
                                      invy_p[:, 2 * mt:2 * mt + 1])

              # ---------------- stage 2: attention ----------------
              with ExitStack() as s2:
                  pTp = s2.enter_context(tc.tile_pool(name=f"pTp{_rep}", bufs=MT + 1))
                  sm = s2.enter_context(tc.tile_pool(name=f"sm{_rep}", bufs=3))
                  psA = s2.enter_context(tc.tile_pool(name=f"psA{_rep}", bufs=2, space="PSUM"))

                  for w in range(NWIN):
                      nsl = slice(w * NW, (w + 1) * NW)
                      pT = []
                      # S^T quads: 4 heads row-tiled into one bank-strided psum tile
                      for mt in range(MT):
                          qd = psA.tile([128, 4, 512], F32, name="qd", tag="qd")
                          for h in range(HEADS):
                              nc.tensor.matmul(
                                  out=qd[:, h, 0:NW],
                                  lhsT=k_sb[32 * h:32 * h + 32, mt * 128:(mt + 1) * 128],
                                  rhs=q_sb[32 * h:32 * h + 32, nsl],
                                  start=True, stop=True,
                                  tile_position=(32 * h, 0))
                          p = pTp.tile([128, HEADS, NW], BF16, name="pT", tag="pT")
                          nc.scalar.activation(out=p[:], in_=qd[:, :, 0:NW],
                                               func=AF.Exp,
                                               scale=invy_s[:, 2 * mt:2 * mt + 1])
                          pT.append(p)

                      # PV with fused denominator; heads packed 2-per-bank (col
                      # tiling). The accumulators borrow a drained S^T quad slot
                      # (same pool tag) so psA's 2x4 banks cover everything.
                      pvq = psA.tile([128, 4, 512], F32, name="pvq", tag="qd")
                      for mt in range(MT):
                          for h in range(HEADS):
                              pvt = pvq[:, h // 2, 0:NW]
                              pbase = 0 if h % 2 == 0 else 64
                              nc.tensor.matmul(
                                  out=pvt[pbase:pbase + 33, :],
                                  lhsT=vT_aug[:, mt, h, 0:33],
                                  rhs=pT[mt][:, h, :],
                                  start=(mt == 0), stop=(mt == MT - 1),
                                  tile_position=(0, pbase))
                      # reciprocal of denominators; broadcast across 32
                      # partitions via K=1 ones-matmul into a spare quad slot
                      rbq = psA.tile([128, 4, 512], F32, name="rbq", tag="qd")
                      for h in range(HEADS):
                          pvt = pvq[:, h // 2, 0:NW]
                          pbase = 0 if h % 2 == 0 else 64
                          rd = sm.tile([1, NW], F32R, name="rd", tag="rd")
                          nc.vector.reciprocal(rd[:], pvt[pbase + 32:pbase + 33, :])
                          nc.tensor.matmul(out=rbq[0:32, h, 0:NW],
                                           lhsT=ones_row[0:1, 0:32],
                                           rhs=rd[0:1, :],
                                           start=True, stop=True)
                          un = sm.tile([32, NW], F32, name="un", tag="un")
                          nc.vector.tensor_copy(un[:], pvt[pbase:pbase + 32, :])
                          nc.vector.tensor_mul(attn_sb[32 * h:32 * h + 32, nsl],
                                               un[:], rbq[0:32, h, 0:NW])

              # ---------------- stage 3: output projection ----------------
              with ExitStack() as s3:
                  psP = s3.enter_context(tc.tile_pool(name=f"psP{_rep}", bufs=2, space="PSUM"))
                  for ct in range(2):
                      for j in range(NWIN):
                          sl = slice(j * NW, (j + 1) * NW)
                          ps = psP.tile([128, NW], F32, name="pj", tag="pj")
                          nc.tensor.matmul(out=ps[:],
                                           lhsT=pw_sb[:, ct * 128:(ct + 1) * 128],
                                           rhs=attn_sb[:, sl], start=True, stop=True)
                          nc.vector.tensor_scalar_add(o_sb[:, ct, sl], ps[:],
                                                      pb_sb[:, ct:ct + 1])
                  for ct in range(2):
                      nc.sync.dma_start(o_d[ct], o_sb[:, ct, :])

    nc.compile()
    return nc


class _Executor:
    """Compile once; run the SPMD kernel via PJRT/axon on 8 cores."""

    def __init__(self, reps=1):
        import jax
        from jax.sharding import Mesh, PartitionSpec
        from jax.experimental.shard_map import shard_map

        from concourse import bass2jax, mybir

        self.nc = _build_module(reps=reps)
        bass2jax.install_neuronx_cc_hook()

        partition_name = (self.nc.partition_id_tensor.name
                          if self.nc.partition_id_tensor else None)
        in_names, out_names, out_avals, zero_outs = [], [], [], []
        for alloc in self.nc.m.functions[0].allocations:
            if not isinstance(alloc, mybir.MemoryLocationSet):
                continue
            name = alloc.memorylocations[0].name
            if alloc.kind == "ExternalInput":
                if name != partition_name:
                    in_names.append(name)
            elif alloc.kind == "ExternalOutput":
                out_names.append(name)
                shape = tuple(alloc.tensor_shape)
                dtype = mybir.dt.np(alloc.dtype)
                out_avals.append(jax.core.ShapedArray(shape, dtype))
                zero_outs.append(np.zeros(shape, dtype))
        self.in_names, self.out_names = in_names, out_names
        self.out_avals, self.zero_outs = out_avals, zero_outs
        n_params, n_outs = len(in_names), len(out_names)
        all_names = list(in_names + out_names)
        if partition_name is not None:
            all_names.append(partition_name)
        all_names = tuple(all_names)

        def make_body(reps):
            def _body(*args):
                operands = list(args)
                if partition_name is not None:
                    operands.append(bass2jax.partition_id_tensor())
                return tuple(bass2jax._bass_exec_p.bind(
                    *operands,
                    out_avals=tuple(out_avals),
                    in_names=all_names,
                    out_names=tuple(out_names),
                    lowering_input_output_aliases=(),
                    sim_require_finite=False,
                    sim_require_nnan=False,
                    nc=self.nc,
                ))
            return _body

        devices = jax.devices()[:N_CORES]
        self.mesh = Mesh(np.asarray(devices), ("core",))
        in_specs = (PartitionSpec("core"),) * (n_params + n_outs)
        out_specs = (PartitionSpec("core"),) * n_outs
        self._jits = {}

        def get_jit(reps):
            if reps not in self._jits:
                self._jits[reps] = jax.jit(
                    shard_map(make_body(reps), mesh=self.mesh,
                              in_specs=in_specs, out_specs=out_specs,
                              check_rep=False),
                    keep_unused=True)
            return self._jits[reps]

        self._get_jit = get_jit

    def _concat_args(self, in_maps):
        cat = [np.concatenate([np.asarray(m[n]) for m in in_maps], axis=0)
               for n in self.in_names]
        cat += [np.concatenate([z] * len(in_maps), axis=0)
                for z in self.zero_outs]
        return cat

    def run(self, in_maps, reps=1):
        args = self._concat_args(in_maps)
        outs = self._get_jit(reps)(*args)
        n = len(in_maps)
        return [
            {name: np.asarray(outs[i]).reshape(n, *self.out_avals[i].shape)[c]
             for i, name in enumerate(self.out_names)}
            for c in range(n)
        ]

    def timed(self, in_maps, reps):
        """Returns wall seconds for a jitted call executing `reps` chained
        kernel executions (first call compiles; call twice)."""
        import time
        import jax
        args = self._concat_args(in_maps)
        fn = self._get_jit(reps)
        outs = fn(*args)  # warm/compile
        jax.block_until_ready(outs)
        best = float("inf")
        for _ in range(3):
            t0 = time.perf_counter()
            outs = fn(*args)
            jax.block_until_ready(outs)
            best = min(best, time.perf_counter() - t0)
        return best


def _get_executor():
    global _EXEC
    if _EXEC is None:
        _EXEC = _Executor()
    return _EXEC


def _make_in_maps(inputs):
    x = np.ascontiguousarray(np.asarray(inputs["x"], np.float32)).reshape(B, CQ, N)
    y = np.ascontiguousarray(np.asarray(inputs["y"], np.float32)).reshape(B, CKV, N)
    q_w = np.asarray(inputs["q_w"], np.float32)
    kv_w = np.asarray(inputs["kv_w"], np.float32)
    qn = np.asarray(inputs["q_norm_w"], np.float32)
    kvn = np.asarray(inputs["kv_norm_w"], np.float32)
    q_b = np.asarray(inputs["q_b"], np.float32)
    kv_b = np.asarray(inputs["kv_b"], np.float32)
    proj_w = np.asarray(inputs["proj_w"], np.float32)
    proj_b = np.asarray(inputs["proj_b"], np.float32)

    qwT = np.ascontiguousarray((q_w * qn[None, :]).T).reshape(2, 128, 128)
    kvwT = np.ascontiguousarray((kv_w * kvn[None, :]).T).reshape(4, 128, 256)
    pwT = np.ascontiguousarray(proj_w.T)
    pb_eff = proj_b + proj_w @ kv_b[128:]
    shared = {
        "qwT": qwT, "kvwT": kvwT, "pwT": pwT,
        "qb": q_b.reshape(128, 1),
        "pb": pb_eff.astype(np.float32).reshape(2, 128, 1),
        "ones": np.ones((128, 1), np.float32),
    }
    in_maps = []
    for c in range(N_CORES):
        b, half = c // 2, c % 2
        xb = np.ascontiguousarray(
            x[b][:, half * NH:(half + 1) * NH]).reshape(2, 128, NH)
        yb = np.ascontiguousarray(y[b]).reshape(4, 128, N)
        in_maps.append({"xb": xb, "yb": yb, **shared})
    return in_maps


def kernel(**inputs):
    ex = _get_executor()
    res = ex.run(_make_in_maps(inputs))
    out = np.empty((B, CQ, N), np.float32)
    for c in range(N_CORES):
        b, half = c // 2, c % 2
        out[b][:, half * NH:(half + 1) * NH] = res[c]["o"].reshape(CQ, NH)
    return out.reshape(B, CQ, 48, 48)
